# revision 47
# baseline (speedup 1.0000x reference)
"""Trainium2 Bass kernel for cross-attention (b=4, nq=2048, nkv=1024,
qdim=1024, cdim=768, heads=16, dim_head=64).

Sharding: 8 cores = batch(4) x nq-half(2). Each core computes a disjoint
[1024, 1024] slice of the output; no collectives needed.

Host-side prep (part of kernel()):
  - KV compaction: the per-key mask drops ~half the keys. Gather the
    unmasked context rows per batch, pad with zeros to a multiple of 128
    (NKVP = nkc*128 kv positions). softmax over masked entries ==
    softmax over the unmasked subset, so this is exact.
  - x and ctx are uploaded pre-transposed (xt = x^T, ctx_t = ctx^T), which
    removes all PE transpose work on device.
  - mask64 ([128, nkc*64], bf16): per-kv-chunk validity column-broadcast
    to 64 cols; used as matmul weights for the softmax rowsum so padding
    rows are excluded. Pad rows of ctx are zero so V pad rows are zero.

Per-core device algorithm:
  KT = Wk^T @ CT   [inner, NKVP]   (inner chunk p holds heads 2p, 2p+1)
  V  = CT^T @ Wv   [NKVP, inner]   (pad rows zero by construction)
  QT = Wq^T @ XT   [inner, nq]     (single pass, all 8 Wq chunks resident)
  per head-pair p, per q-half hf, per kv-chunk c:
    S^T = K_h @ Q_h^T              (row-tiled K=64: 2 heads concurrent)
    ES  = exp(SCALE * S^T)         (ScalarE, PSUM->SBUF, bf16)
    OT_acc  += V_h^T @ ES          (col-tiled M=64: 2 heads concurrent)
    rs_acc  += mask64^T @ ES       (col-tiled M=64, excludes pad rows)
  rT = 1/rs ; OT = OT_acc * rT (bf16)
  out = (OT^T stacked) @ Wo + 1 x bo  (bf16 matmuls, bias via K=1 matmul)

Scores are O(1) by construction so unshifted exp is safe.
"""

import numpy as np
from contextlib import ExitStack

import ml_dtypes

import concourse.bass as bass
import concourse.mybir as mybir
import concourse.tile as tile
from concourse import bacc
from concourse.bass_utils import run_bass_kernel_spmd

F32 = mybir.dt.float32
F32R = mybir.dt.float32r
BF16 = mybir.dt.bfloat16
AF = mybir.ActivationFunctionType
NPBF16 = ml_dtypes.bfloat16

NQ = 1024      # queries per core
NKV = 1024
QD = 1024
CD = 768
H = 16
D = 64
INNER = 1024
SCALE = D ** -0.5
P = 128
NQC = NQ // P      # 8 nq chunks
QDC = QD // P      # 8
CDC = CD // P      # 6
HP = H // 2        # 8 head pairs


def R(ap):
    return ap.bitcast(F32R)


def _emit(tc, io, nkc, stages=("proj", "attn", "out")):
    nc = tc.nc
    xt_d, ctx_t, mask64_d, wq_d, wk_d, wv_d, wo_d, bo_d, out_d = io
    NKVP = nkc * P
    do_proj = "proj" in stages
    attn_lv = (4 if "attn" in stages else 3 if "attn3" in stages
               else 2 if "attn2" in stages else 1 if "attn1" in stages else 0)
    do_out = "out" in stages
    has_bias = "nobias" not in stages

    with ExitStack() as top:
        # ---- flat pools: everything resident, no pool-boundary barriers ----
        const = top.enter_context(tc.tile_pool(name="const", bufs=1))
        big = top.enter_context(tc.tile_pool(name="big", bufs=1))

        ones = const.tile([1, P], BF16, tag="ones")
        nc.vector.memset(ones[:], 1.0)
        mask64 = const.tile([P, nkc * D], BF16, tag="mask64")
        nc.sync.dma_start(out=mask64[:], in_=mask64_d)

        ot = big.tile([P, QDC * NQ], BF16, tag="ot")     # O^T: chunk k cols k*NQ..
        qt = big.tile([P, HP * NQ], BF16, tag="qt")      # Q^T: chunk p cols p*NQ..
        kt = big.tile([P, HP * NKVP], BF16, tag="kt")
        vt = big.tile([P, nkc * INNER], BF16, tag="vt")  # V: chunk c cols c*INNER..

        # ---- input loads (few big DMAs, split across both HWDGE queues;
        # ordered so K-proj deps (ct on sync, wk on scalar) land first) ----
        ct = big.tile([P, CDC * NKVP], BF16, tag="ct")
        ct3 = ct.rearrange("p (k n) -> p k n", n=NKVP)
        wkt = big.tile([P, CDC * INNER], BF16, tag="wk")
        wk3 = wkt.rearrange("p (k n) -> p k n", n=INNER)
        # two halves so the first K-proj matmuls start while the second
        # half of ct/wk is still streaming in
        hc = CDC // 2
        for h in range(2):
            k0, k1 = h * hc, (h + 1) * hc
            nc.sync.dma_start(
                out=ct3[:, k0:k1, :],
                in_=ctx_t[k0 * P:k1 * P, :].rearrange("(k p) n -> p k n", p=P))
            nc.scalar.dma_start(
                out=wk3[:, k0:k1, :],
                in_=wk_d[k0 * P:k1 * P, :].rearrange("(k p) n -> p k n", p=P))
        wk = [wk3[:, k, :] for k in range(CDC)]
        xt = big.tile([P, QDC * NQ], BF16, tag="xt")
        xt3 = xt.rearrange("p (k n) -> p k n", n=NQ)
        nc.sync.dma_start(out=xt3[:], in_=xt_d.rearrange("(k p) n -> p k n", p=P))
        wqt = big.tile([P, QDC * INNER], BF16, tag="wq")
        wq3 = wqt.rearrange("p (k n) -> p k n", n=INNER)
        nc.scalar.dma_start(out=wq3[:], in_=wq_d.rearrange("(k p) n -> p k n", p=P))
        wq = [wq3[:, k, :] for k in range(QDC)]
        wvt = big.tile([P, CDC * INNER], BF16, tag="wv")
        wv3 = wvt.rearrange("p (k n) -> p k n", n=INNER)
        nc.sync.dma_start(out=wv3[:], in_=wv_d.rearrange("(k p) n -> p k n", p=P))
        wv = [wv3[:, k, :] for k in range(CDC)]
        wo = big.tile([P, QDC * QD], BF16, tag="wo")
        wo3 = wo.rearrange("p (k n) -> p k n", n=QD)
        nc.scalar.dma_start(out=wo3[:], in_=wo_d.rearrange("(k p) n -> p k n", p=P))
        bo_t = const.tile([1, QD], BF16, tag="bo")
        nc.sync.dma_start(out=bo_t[:], in_=bo_d[:].rearrange("(o n) -> o n", o=1))

        # ---- K projection: KT[inner, kv] (psum must not cross 2KB banks) ----
        ksplits = [(lo, min(lo + 512, NKVP)) for lo in range(0, NKVP, 512)]
        proj_ctx = ExitStack()
        pj_ps = proj_ctx.enter_context(tc.tile_pool(name="pj_ps", bufs=2, space="PSUM"))
        for p in range(HP if do_proj else 0):
            ps = pj_ps.tile([P, NKVP], F32, tag="pj", name=f"pjk{p}")
            for k in range(CDC):
                for (lo, hi) in ksplits:
                    nc.tensor.matmul(
                        ps[:, lo:hi], wk[k][:, p * P:(p + 1) * P],
                        ct[:, k * NKVP + lo: k * NKVP + hi],
                        start=(k == 0), stop=(k == CDC - 1),
                        skip_group_check=True)
            nc.vector.tensor_copy(kt[:, p * NKVP:(p + 1) * NKVP], ps[:])

        # ---- V projection ----
        vq_ps = proj_ctx.enter_context(tc.tile_pool(name="vq_ps", bufs=4, space="PSUM"))
        for c in range(nkc if do_proj else 0):
            for hf in range(2):
                ps = vq_ps.tile([P, 512], F32, tag="vq", name=f"pjv{c}_{hf}")
                for k in range(CDC):
                    nc.tensor.matmul(
                        ps[:], ct[:, k * NKVP + c * P: k * NKVP + (c + 1) * P],
                        wv[k][:, hf * 512:(hf + 1) * 512],
                        start=(k == 0), stop=(k == CDC - 1))
                nc.vector.tensor_copy(
                    vt[:, c * INNER + hf * 512: c * INNER + (hf + 1) * 512],
                    ps[:])

        # ---- Q projection ----
        for p in range(HP if do_proj else 0):
            for hf in range(2):
                ps = vq_ps.tile([P, 512], F32, tag="vq", name=f"pjq{p}_{hf}")
                for k in range(QDC):
                    nc.tensor.matmul(
                        ps[:], wq[k][:, p * P:(p + 1) * P],
                        xt[:, k * NQ + hf * 512: k * NQ + (hf + 1) * 512],
                        start=(k == 0), stop=(k == QDC - 1))
                nc.vector.tensor_copy(
                    qt[:, p * NQ + hf * 512: p * NQ + (hf + 1) * 512], ps[:])

        # ---- attention ----
        proj_ctx.close()
        attn_ctx = ExitStack()
        es_pool = attn_ctx.enter_context(tc.tile_pool(name="esp", bufs=5))
        rt_pool = attn_ctx.enter_context(tc.tile_pool(name="rtp", bufs=2))
        # ps_s=2 + double-buffered po/pr measured faster on HW than ps_s=3 +
        # single po/pr (250.7us vs 254.9us), opposite of TimelineSim's
        # prediction — the pair-boundary accumulator WAR matters more on
        # silicon than the S-pipeline depth
        ps_s = attn_ctx.enter_context(tc.tile_pool(name="ps_s", bufs=2, space="PSUM"))
        ps_o = attn_ctx.enter_context(tc.tile_pool(name="ps_o", bufs=2, space="PSUM"))
        ps_r = attn_ctx.enter_context(tc.tile_pool(name="ps_r", bufs=2, space="PSUM"))
        for p in range(HP if attn_lv else 0):
            for hf in range(2):
                po = ps_o.tile([P, 512], F32, tag="po", name=f"po{p}_{hf}")
                pr = ps_r.tile([P, 512], F32, tag="pr", name=f"pr{p}_{hf}")
                esq = []

                def S(c, p=p, hf=hf, esq=esq):
                    # S^T for both heads of the pair (row-tiled K=64):
                    # head h -> cols 0:512, head h' -> cols 512:1024
                    ps = ps_s.tile([P, NQ], F32, tag="ss", name=f"ss{p}_{hf}_{c}")
                    for hh in range(2):
                        nc.tensor.matmul(
                            ps[:, hh * 512:(hh + 1) * 512],
                            kt[hh * D:(hh + 1) * D,
                               p * NKVP + c * P: p * NKVP + (c + 1) * P],
                            qt[hh * D:(hh + 1) * D,
                               p * NQ + hf * 512: p * NQ + (hf + 1) * 512],
                            start=True, stop=True,
                            tile_position=(hh * D, 0))
                    if attn_lv < 2:
                        return
                    es = es_pool.tile([P, NQ], BF16, tag="es",
                                      name=f"es{p}_{hf}_{c}")
                    nc.scalar.activation(es[:], ps[:], AF.Exp, scale=float(SCALE))
                    esq.append(es)

                # one-chunk S lookahead: S(c+1) is emitted before AV(c) so
                # the in-order PE queue does not wait out exp(c)'s latency
                S(0)
                for c in range(nkc):
                    if c + 1 < nkc:
                        S(c + 1)
                    if attn_lv < 3:
                        continue
                    es = esq[c]
                    for hh in range(2):
                        h = 2 * p + hh
                        esl = es[:, hh * 512:(hh + 1) * 512]
                        nc.tensor.matmul(
                            po[hh * D:(hh + 1) * D, :],
                            vt[:, c * INNER + h * D: c * INNER + (h + 1) * D],
                            esl,
                            start=(c == 0), stop=(c == nkc - 1),
                            tile_position=(0, hh * D),
                            skip_group_check=True)
                        if attn_lv >= 4:
                            nc.tensor.matmul(
                                pr[hh * D:(hh + 1) * D, :],
                                mask64[:, c * D:(c + 1) * D], esl,
                                start=(c == 0), stop=(c == nkc - 1),
                                tile_position=(0, hh * D),
                                skip_group_check=True)
                if attn_lv < 4:
                    continue
                # epilogue: normalize this (pair, nq-half) slice
                rt = rt_pool.tile([P, 512], F32, tag="rt", name=f"rt{p}_{hf}")
                with nc.allow_low_precision(reason="softmax reciprocal"):
                    nc.vector.reciprocal(rt[:], pr[:])
                nc.vector.tensor_mul(
                    ot[:, p * NQ + hf * 512: p * NQ + (hf + 1) * 512],
                    po[:], rt[:])


        # ---- output projection ----
        attn_ctx.close()
        out_ps = top.enter_context(tc.tile_pool(name="out_ps", bufs=6, space="PSUM"))
        out_sb = top.enter_context(tc.tile_pool(name="out_sb", bufs=3))
        for m in range(NQC if do_out else 0):
            sb = out_sb.tile([P, QD], BF16, tag="osb", name=f"osb{m}")
            for n in range(2):
                ps = out_ps.tile([P, 512], F32, tag="ops", name=f"ops{m}_{n}")
                for k in range(QDC):
                    nc.tensor.matmul(
                        ps[:],
                        ot[:, k * NQ + m * P: k * NQ + (m + 1) * P],
                        wo3[:, k, n * 512:(n + 1) * 512],
                        start=(k == 0),
                        stop=(k == QDC - 1) and not has_bias,
                        skip_group_check=True)
                if has_bias:
                    nc.tensor.matmul(
                        ps[:], ones[0:1, 0:P],
                        bo_t[0:1, n * 512:(n + 1) * 512],
                        start=False, stop=True, skip_group_check=True)
                nc.vector.tensor_copy(sb[:, n * 512:(n + 1) * 512], ps[:])
            nc.sync.dma_start(out=out_d[m * P:(m + 1) * P, :], in_=sb[:])

_CACHED = {}


def _build(iters=1, loop=1, nkc=5, stages=("proj", "attn", "out"), staggered=True):
    """Build the program. `iters` unrolls the body in the instruction stream;
    `loop` wraps it in an on-device hardware loop (constant program size) —
    used by test.py to measure per-body device time as a slope. `nkc` is the
    number of 128-row kv chunks after mask compaction. `stages` restricts the
    emitted phases (timing probes only — output is garbage unless full)."""
    key = (iters, loop, nkc, tuple(stages), staggered)
    if key in _CACHED:
        return _CACHED[key]
    NKVP = nkc * P
    nc = bacc.Bacc("TRN2", debug=False, target_bir_lowering=False)
    xt = nc.dram_tensor("xt", [QD, NQ], BF16, kind="ExternalInput").ap()
    ctx_t = nc.dram_tensor("ctx_t", [CD, NKVP], BF16, kind="ExternalInput").ap()
    mask64 = nc.dram_tensor("mask64", [P, nkc * D], BF16,
                            kind="ExternalInput").ap()
    wq_d = nc.dram_tensor("wq", [QD, INNER], BF16, kind="ExternalInput").ap()
    wk_d = nc.dram_tensor("wk", [CD, INNER], BF16, kind="ExternalInput").ap()
    wv_d = nc.dram_tensor("wv", [CD, INNER], BF16, kind="ExternalInput").ap()
    wo_d = nc.dram_tensor("wo", [INNER, QD], BF16, kind="ExternalInput").ap()
    bo_d = nc.dram_tensor("bo", [QD], BF16, kind="ExternalInput").ap()
    out_d = nc.dram_tensor("out", [NQ, QD], BF16, kind="ExternalOutput").ap()
    io = (xt, ctx_t, mask64, wq_d, wk_d, wv_d, wo_d, bo_d, out_d)
    with tile.TileContext(nc) as tc:
        if loop > 1:
            with tc.For_i(0, loop, 1, staggered_reset=staggered,
                          hint_engines=(mybir.EngineType.PE,)):
                for _ in range(iters):
                    _emit(tc, io, nkc, stages)
        else:
            for _ in range(iters):
                _emit(tc, io, nkc, stages)
    nc.compile()
    _CACHED[key] = nc
    return nc


def make_in_maps(x, context, mask, Wq, Wk, Wv, Wo, bo):
    x = np.asarray(x, dtype=np.float32)
    context = np.asarray(context, dtype=np.float32)
    mask_b = np.asarray(mask).astype(bool)
    Wq = np.ascontiguousarray(np.asarray(Wq, dtype=np.float32)).astype(NPBF16)
    Wk = np.ascontiguousarray(np.asarray(Wk, dtype=np.float32)).astype(NPBF16)
    Wv = np.ascontiguousarray(np.asarray(Wv, dtype=np.float32)).astype(NPBF16)
    Wo = np.ascontiguousarray(np.asarray(Wo, dtype=np.float32)).astype(NPBF16)
    bo = np.ascontiguousarray(np.asarray(bo, dtype=np.float32)).astype(NPBF16)

    counts = mask_b.sum(axis=1)
    n_max = max(int(counts.max()), 1)
    nkc = (n_max + P - 1) // P
    NKVP = nkc * P

    # a fully-masked batch reduces to uniform attention over all keys:
    # emulate exactly by sending the full context unmasked with Wq zeroed
    # (s = 0 -> softmax uniform), matching the reference's -inf softmax
    if (counts == 0).any():
        nkc = NKV // P
        NKVP = nkc * P

    ctx_ts, m64s, wq_zero = [], [], []
    for b in range(4):
        idx = np.nonzero(mask_b[b])[0]
        n = len(idx)
        wq_zero.append(n == 0)
        ctx_c = np.zeros((NKVP, CD), np.float32)
        if n:
            ctx_c[:n] = context[b][idx]
        else:
            n = NKV
            ctx_c[:n] = context[b]
        ctx_ts.append(np.ascontiguousarray(ctx_c.T).astype(NPBF16))
        valid = (np.arange(NKVP) < n).reshape(nkc, P)      # [c, p]
        m64 = np.repeat(valid.T[:, :, None], D, axis=2)    # [p, c, 64]
        m64s.append(np.ascontiguousarray(
            m64.reshape(P, nkc * D)).astype(NPBF16))

    in_maps = []
    for b in range(4):
        for qh in range(2):
            in_maps.append({
                "xt": np.ascontiguousarray(
                    x[b, qh * NQ:(qh + 1) * NQ, :].T).astype(NPBF16),
                "ctx_t": ctx_ts[b],
                "mask64": m64s[b],
                "wq": np.zeros_like(Wq) if wq_zero[b] else Wq,
                "wk": Wk, "wv": Wv, "wo": Wo, "bo": bo,
            })
    return in_maps, nkc


def run_sharded(x, context, mask, Wq, Wk, Wv, Wo, bo, trace=False, **kw):
    in_maps, nkc = make_in_maps(x, context, mask, Wq, Wk, Wv, Wo, bo)
    stages = ("proj", "attn", "out") + (
        () if np.asarray(bo).any() else ("nobias",))
    nc = _build(nkc=nkc, stages=stages)
    res = run_bass_kernel_spmd(nc, in_maps, list(range(8)), trace=trace, **kw)
    out = np.empty((4, 2 * NQ, QD), dtype=np.float32)
    for i in range(8):
        b, qh = divmod(i, 2)
        out[b, qh * NQ:(qh + 1) * NQ, :] = res.results[i]["out"].astype(np.float32)
    return out, res


def kernel(x, context, mask, Wq, Wk, Wv, Wo, bo):
    out, _ = run_sharded(x, context, mask, Wq, Wk, Wv, Wo, bo, trace=False)
    return out



# revision 48
# speedup vs baseline: 1.0242x; 1.0242x over previous
"""Trainium2 Bass kernel for cross-attention (b=4, nq=2048, nkv=1024,
qdim=1024, cdim=768, heads=16, dim_head=64).

Sharding: 8 cores = batch(4) x nq-half(2). Each core computes a disjoint
[1024, 1024] slice of the output; no collectives needed.

Host-side prep (part of kernel()):
  - KV compaction: the per-key mask drops ~half the keys. Gather the
    unmasked context rows per batch, pad with zeros to a multiple of 128
    (NKVP = nkc*128 kv positions). softmax over masked entries ==
    softmax over the unmasked subset, so this is exact.
  - x and ctx are uploaded pre-transposed (xt = x^T, ctx_t = ctx^T), which
    removes all PE transpose work on device.
  - mask64 ([128, nkc*64], bf16): per-kv-chunk validity column-broadcast
    to 64 cols; used as matmul weights for the softmax rowsum so padding
    rows are excluded. Pad rows of ctx are zero so V pad rows are zero.

Per-core device algorithm:
  KT = Wk^T @ CT   [inner, NKVP]   (inner chunk p holds heads 2p, 2p+1)
  V  = CT^T @ Wv   [NKVP, inner]   (pad rows zero by construction)
  QT = Wq^T @ XT   [inner, nq]     (single pass, all 8 Wq chunks resident)
  per head-pair p, per q-half hf, per kv-chunk c:
    S^T = K_h @ Q_h^T              (row-tiled K=64: 2 heads concurrent)
    ES  = exp(SCALE * S^T)         (ScalarE, PSUM->SBUF, bf16)
    OT_acc  += V_h^T @ ES          (col-tiled M=64: 2 heads concurrent)
    rs_acc  += mask64^T @ ES       (col-tiled M=64, excludes pad rows)
  rT = 1/rs ; OT = OT_acc * rT (bf16)
  out = (OT^T stacked) @ Wo + 1 x bo  (bf16 matmuls, bias via K=1 matmul)

Scores are O(1) by construction so unshifted exp is safe.
"""

import numpy as np
from contextlib import ExitStack

import ml_dtypes

import concourse.bass as bass
import concourse.mybir as mybir
import concourse.tile as tile
from concourse import bacc
from concourse.bass_utils import run_bass_kernel_spmd

F32 = mybir.dt.float32
F32R = mybir.dt.float32r
BF16 = mybir.dt.bfloat16
AF = mybir.ActivationFunctionType
NPBF16 = ml_dtypes.bfloat16

NQ = 1024      # queries per core
NKV = 1024
QD = 1024
CD = 768
H = 16
D = 64
INNER = 1024
SCALE = D ** -0.5
P = 128
NQC = NQ // P      # 8 nq chunks
QDC = QD // P      # 8
CDC = CD // P      # 6
HP = H // 2        # 8 head pairs


def R(ap):
    return ap.bitcast(F32R)


def _emit(tc, io, nkc, stages=("proj", "attn", "out")):
    nc = tc.nc
    xt_d, ctx_t, mask64_d, wq_d, wk_d, wv_d, wo_d, bo_d, out_d = io
    NKVP = nkc * P
    do_proj = "proj" in stages
    attn_lv = (4 if "attn" in stages else 3 if "attn3" in stages
               else 2 if "attn2" in stages else 1 if "attn1" in stages else 0)
    do_out = "out" in stages
    has_bias = "nobias" not in stages

    with ExitStack() as top:
        # ---- flat pools: everything resident, no pool-boundary barriers ----
        const = top.enter_context(tc.tile_pool(name="const", bufs=1))
        big = top.enter_context(tc.tile_pool(name="big", bufs=1))

        ones = const.tile([1, P], BF16, tag="ones")
        nc.vector.memset(ones[:], 1.0)
        mask64 = const.tile([P, nkc * D], BF16, tag="mask64")
        nc.sync.dma_start(out=mask64[:], in_=mask64_d)

        ot = big.tile([P, QDC * NQ], BF16, tag="ot")     # O^T: chunk k cols k*NQ..
        qt = big.tile([P, HP * NQ], BF16, tag="qt")      # Q^T: chunk p cols p*NQ..
        kt = big.tile([P, HP * NKVP], BF16, tag="kt")
        vt = big.tile([P, nkc * INNER], BF16, tag="vt")  # V: chunk c cols c*INNER..

        # ---- input loads (few big DMAs, split across both HWDGE queues;
        # ordered so K-proj deps (ct on sync, wk on scalar) land first) ----
        ct = big.tile([P, CDC * NKVP], BF16, tag="ct")
        ct3 = ct.rearrange("p (k n) -> p k n", n=NKVP)
        nc.sync.dma_start(out=ct3[:], in_=ctx_t.rearrange("(k p) n -> p k n", p=P))
        wkt = big.tile([P, CDC * INNER], BF16, tag="wk")
        wk3 = wkt.rearrange("p (k n) -> p k n", n=INNER)
        nc.scalar.dma_start(out=wk3[:], in_=wk_d.rearrange("(k p) n -> p k n", p=P))
        wk = [wk3[:, k, :] for k in range(CDC)]
        xt = big.tile([P, QDC * NQ], BF16, tag="xt")
        xt3 = xt.rearrange("p (k n) -> p k n", n=NQ)
        nc.sync.dma_start(out=xt3[:], in_=xt_d.rearrange("(k p) n -> p k n", p=P))
        wqt = big.tile([P, QDC * INNER], BF16, tag="wq")
        wq3 = wqt.rearrange("p (k n) -> p k n", n=INNER)
        nc.scalar.dma_start(out=wq3[:], in_=wq_d.rearrange("(k p) n -> p k n", p=P))
        wq = [wq3[:, k, :] for k in range(QDC)]
        wvt = big.tile([P, CDC * INNER], BF16, tag="wv")
        wv3 = wvt.rearrange("p (k n) -> p k n", n=INNER)
        nc.sync.dma_start(out=wv3[:], in_=wv_d.rearrange("(k p) n -> p k n", p=P))
        wv = [wv3[:, k, :] for k in range(CDC)]
        wo = big.tile([P, QDC * QD], BF16, tag="wo")
        wo3 = wo.rearrange("p (k n) -> p k n", n=QD)
        nc.scalar.dma_start(out=wo3[:], in_=wo_d.rearrange("(k p) n -> p k n", p=P))
        bo_t = const.tile([1, QD], BF16, tag="bo")
        nc.sync.dma_start(out=bo_t[:], in_=bo_d[:].rearrange("(o n) -> o n", o=1))

        # ---- K projection: KT[inner, kv] (psum must not cross 2KB banks) ----
        ksplits = [(lo, min(lo + 512, NKVP)) for lo in range(0, NKVP, 512)]
        proj_ctx = ExitStack()
        pj_ps = proj_ctx.enter_context(tc.tile_pool(name="pj_ps", bufs=2, space="PSUM"))
        for p in range(HP if do_proj else 0):
            ps = pj_ps.tile([P, NKVP], F32, tag="pj", name=f"pjk{p}")
            for k in range(CDC):
                for (lo, hi) in ksplits:
                    nc.tensor.matmul(
                        ps[:, lo:hi], wk[k][:, p * P:(p + 1) * P],
                        ct[:, k * NKVP + lo: k * NKVP + hi],
                        start=(k == 0), stop=(k == CDC - 1),
                        skip_group_check=True)
            nc.vector.tensor_copy(kt[:, p * NKVP:(p + 1) * NKVP], ps[:])

        # ---- V projection ----
        vq_ps = proj_ctx.enter_context(tc.tile_pool(name="vq_ps", bufs=4, space="PSUM"))
        for c in range(nkc if do_proj else 0):
            for hf in range(2):
                ps = vq_ps.tile([P, 512], F32, tag="vq", name=f"pjv{c}_{hf}")
                for k in range(CDC):
                    nc.tensor.matmul(
                        ps[:], ct[:, k * NKVP + c * P: k * NKVP + (c + 1) * P],
                        wv[k][:, hf * 512:(hf + 1) * 512],
                        start=(k == 0), stop=(k == CDC - 1))
                nc.vector.tensor_copy(
                    vt[:, c * INNER + hf * 512: c * INNER + (hf + 1) * 512],
                    ps[:])

        # ---- Q projection ----
        for p in range(HP if do_proj else 0):
            for hf in range(2):
                ps = vq_ps.tile([P, 512], F32, tag="vq", name=f"pjq{p}_{hf}")
                for k in range(QDC):
                    nc.tensor.matmul(
                        ps[:], wq[k][:, p * P:(p + 1) * P],
                        xt[:, k * NQ + hf * 512: k * NQ + (hf + 1) * 512],
                        start=(k == 0), stop=(k == QDC - 1))
                nc.vector.tensor_copy(
                    qt[:, p * NQ + hf * 512: p * NQ + (hf + 1) * 512], ps[:])

        # ---- attention ----
        proj_ctx.close()
        attn_ctx = ExitStack()
        es_pool = attn_ctx.enter_context(tc.tile_pool(name="esp", bufs=5))
        rt_pool = attn_ctx.enter_context(tc.tile_pool(name="rtp", bufs=2))
        # ps_s=2 + double-buffered po/pr measured faster on HW than ps_s=3 +
        # single po/pr (250.7us vs 254.9us), opposite of TimelineSim's
        # prediction — the pair-boundary accumulator WAR matters more on
        # silicon than the S-pipeline depth
        ps_s = attn_ctx.enter_context(tc.tile_pool(name="ps_s", bufs=2, space="PSUM"))
        ps_o = attn_ctx.enter_context(tc.tile_pool(name="ps_o", bufs=2, space="PSUM"))
        ps_r = attn_ctx.enter_context(tc.tile_pool(name="ps_r", bufs=2, space="PSUM"))
        for p in range(HP if attn_lv else 0):
            for hf in range(2):
                po = ps_o.tile([P, 512], F32, tag="po", name=f"po{p}_{hf}")
                pr = ps_r.tile([P, 512], F32, tag="pr", name=f"pr{p}_{hf}")
                esq = []

                def S(c, p=p, hf=hf, esq=esq):
                    # S^T for both heads of the pair (row-tiled K=64):
                    # head h -> cols 0:512, head h' -> cols 512:1024
                    ps = ps_s.tile([P, NQ], F32, tag="ss", name=f"ss{p}_{hf}_{c}")
                    for hh in range(2):
                        nc.tensor.matmul(
                            ps[:, hh * 512:(hh + 1) * 512],
                            kt[hh * D:(hh + 1) * D,
                               p * NKVP + c * P: p * NKVP + (c + 1) * P],
                            qt[hh * D:(hh + 1) * D,
                               p * NQ + hf * 512: p * NQ + (hf + 1) * 512],
                            start=True, stop=True,
                            tile_position=(hh * D, 0))
                    if attn_lv < 2:
                        return
                    es = es_pool.tile([P, NQ], BF16, tag="es",
                                      name=f"es{p}_{hf}_{c}")
                    nc.scalar.activation(es[:], ps[:], AF.Exp, scale=float(SCALE))
                    esq.append(es)

                # one-chunk S lookahead: S(c+1) is emitted before AV(c) so
                # the in-order PE queue does not wait out exp(c)'s latency
                S(0)
                for c in range(nkc):
                    if c + 1 < nkc:
                        S(c + 1)
                    if attn_lv < 3:
                        continue
                    es = esq[c]
                    for hh in range(2):
                        h = 2 * p + hh
                        esl = es[:, hh * 512:(hh + 1) * 512]
                        nc.tensor.matmul(
                            po[hh * D:(hh + 1) * D, :],
                            vt[:, c * INNER + h * D: c * INNER + (h + 1) * D],
                            esl,
                            start=(c == 0), stop=(c == nkc - 1),
                            tile_position=(0, hh * D),
                            skip_group_check=True)
                        if attn_lv >= 4:
                            nc.tensor.matmul(
                                pr[hh * D:(hh + 1) * D, :],
                                mask64[:, c * D:(c + 1) * D], esl,
                                start=(c == 0), stop=(c == nkc - 1),
                                tile_position=(0, hh * D),
                                skip_group_check=True)
                if attn_lv < 4:
                    continue
                # epilogue: normalize this (pair, nq-half) slice
                rt = rt_pool.tile([P, 512], F32, tag="rt", name=f"rt{p}_{hf}")
                with nc.allow_low_precision(reason="softmax reciprocal"):
                    nc.vector.reciprocal(rt[:], pr[:])
                nc.vector.tensor_mul(
                    ot[:, p * NQ + hf * 512: p * NQ + (hf + 1) * 512],
                    po[:], rt[:])


        # ---- output projection ----
        attn_ctx.close()
        out_ps = top.enter_context(tc.tile_pool(name="out_ps", bufs=6, space="PSUM"))
        out_sb = top.enter_context(tc.tile_pool(name="out_sb", bufs=3))
        for m in range(NQC if do_out else 0):
            sb = out_sb.tile([P, QD], BF16, tag="osb", name=f"osb{m}")
            for n in range(2):
                ps = out_ps.tile([P, 512], F32, tag="ops", name=f"ops{m}_{n}")
                for k in range(QDC):
                    nc.tensor.matmul(
                        ps[:],
                        ot[:, k * NQ + m * P: k * NQ + (m + 1) * P],
                        wo3[:, k, n * 512:(n + 1) * 512],
                        start=(k == 0),
                        stop=(k == QDC - 1) and not has_bias,
                        skip_group_check=True)
                if has_bias:
                    nc.tensor.matmul(
                        ps[:], ones[0:1, 0:P],
                        bo_t[0:1, n * 512:(n + 1) * 512],
                        start=False, stop=True, skip_group_check=True)
                nc.vector.tensor_copy(sb[:, n * 512:(n + 1) * 512], ps[:])
            nc.sync.dma_start(out=out_d[m * P:(m + 1) * P, :], in_=sb[:])

_CACHED = {}


def _build(iters=1, loop=1, nkc=5, stages=("proj", "attn", "out"), staggered=True):
    """Build the program. `iters` unrolls the body in the instruction stream;
    `loop` wraps it in an on-device hardware loop (constant program size) —
    used by test.py to measure per-body device time as a slope. `nkc` is the
    number of 128-row kv chunks after mask compaction. `stages` restricts the
    emitted phases (timing probes only — output is garbage unless full)."""
    key = (iters, loop, nkc, tuple(stages), staggered)
    if key in _CACHED:
        return _CACHED[key]
    NKVP = nkc * P
    nc = bacc.Bacc("TRN2", debug=False, target_bir_lowering=False)
    xt = nc.dram_tensor("xt", [QD, NQ], BF16, kind="ExternalInput").ap()
    ctx_t = nc.dram_tensor("ctx_t", [CD, NKVP], BF16, kind="ExternalInput").ap()
    mask64 = nc.dram_tensor("mask64", [P, nkc * D], BF16,
                            kind="ExternalInput").ap()
    wq_d = nc.dram_tensor("wq", [QD, INNER], BF16, kind="ExternalInput").ap()
    wk_d = nc.dram_tensor("wk", [CD, INNER], BF16, kind="ExternalInput").ap()
    wv_d = nc.dram_tensor("wv", [CD, INNER], BF16, kind="ExternalInput").ap()
    wo_d = nc.dram_tensor("wo", [INNER, QD], BF16, kind="ExternalInput").ap()
    bo_d = nc.dram_tensor("bo", [QD], BF16, kind="ExternalInput").ap()
    out_d = nc.dram_tensor("out", [NQ, QD], BF16, kind="ExternalOutput").ap()
    io = (xt, ctx_t, mask64, wq_d, wk_d, wv_d, wo_d, bo_d, out_d)
    with tile.TileContext(nc) as tc:
        if loop > 1:
            with tc.For_i(0, loop, 1, staggered_reset=staggered,
                          hint_engines=(mybir.EngineType.PE,)):
                for _ in range(iters):
                    _emit(tc, io, nkc, stages)
        else:
            for _ in range(iters):
                _emit(tc, io, nkc, stages)
    nc.compile()
    _CACHED[key] = nc
    return nc


def make_in_maps(x, context, mask, Wq, Wk, Wv, Wo, bo):
    x = np.asarray(x, dtype=np.float32)
    context = np.asarray(context, dtype=np.float32)
    mask_b = np.asarray(mask).astype(bool)
    Wq = np.ascontiguousarray(np.asarray(Wq, dtype=np.float32)).astype(NPBF16)
    Wk = np.ascontiguousarray(np.asarray(Wk, dtype=np.float32)).astype(NPBF16)
    Wv = np.ascontiguousarray(np.asarray(Wv, dtype=np.float32)).astype(NPBF16)
    Wo = np.ascontiguousarray(np.asarray(Wo, dtype=np.float32)).astype(NPBF16)
    bo = np.ascontiguousarray(np.asarray(bo, dtype=np.float32)).astype(NPBF16)

    counts = mask_b.sum(axis=1)
    n_max = max(int(counts.max()), 1)
    nkc = (n_max + P - 1) // P
    NKVP = nkc * P

    # a fully-masked batch reduces to uniform attention over all keys:
    # emulate exactly by sending the full context unmasked with Wq zeroed
    # (s = 0 -> softmax uniform), matching the reference's -inf softmax
    if (counts == 0).any():
        nkc = NKV // P
        NKVP = nkc * P

    ctx_ts, m64s, wq_zero = [], [], []
    for b in range(4):
        idx = np.nonzero(mask_b[b])[0]
        n = len(idx)
        wq_zero.append(n == 0)
        ctx_c = np.zeros((NKVP, CD), np.float32)
        if n:
            ctx_c[:n] = context[b][idx]
        else:
            n = NKV
            ctx_c[:n] = context[b]
        ctx_ts.append(np.ascontiguousarray(ctx_c.T).astype(NPBF16))
        valid = (np.arange(NKVP) < n).reshape(nkc, P)      # [c, p]
        m64 = np.repeat(valid.T[:, :, None], D, axis=2)    # [p, c, 64]
        m64s.append(np.ascontiguousarray(
            m64.reshape(P, nkc * D)).astype(NPBF16))

    in_maps = []
    for b in range(4):
        for qh in range(2):
            in_maps.append({
                "xt": np.ascontiguousarray(
                    x[b, qh * NQ:(qh + 1) * NQ, :].T).astype(NPBF16),
                "ctx_t": ctx_ts[b],
                "mask64": m64s[b],
                "wq": np.zeros_like(Wq) if wq_zero[b] else Wq,
                "wk": Wk, "wv": Wv, "wo": Wo, "bo": bo,
            })
    return in_maps, nkc


def run_sharded(x, context, mask, Wq, Wk, Wv, Wo, bo, trace=False, **kw):
    in_maps, nkc = make_in_maps(x, context, mask, Wq, Wk, Wv, Wo, bo)
    stages = ("proj", "attn", "out") + (
        () if np.asarray(bo).any() else ("nobias",))
    nc = _build(nkc=nkc, stages=stages)
    res = run_bass_kernel_spmd(nc, in_maps, list(range(8)), trace=trace, **kw)
    out = np.empty((4, 2 * NQ, QD), dtype=np.float32)
    for i in range(8):
        b, qh = divmod(i, 2)
        out[b, qh * NQ:(qh + 1) * NQ, :] = res.results[i]["out"].astype(np.float32)
    return out, res


def kernel(x, context, mask, Wq, Wk, Wv, Wo, bo):
    out, _ = run_sharded(x, context, mask, Wq, Wk, Wv, Wo, bo, trace=False)
    return out

